# revision 8
# baseline (speedup 1.0000x reference)
# Multi-head causal self-attention (B=2, S=2048, D=1024, H=16, Dh=64) on 8
# Trainium2 NeuronCores.
#
# Sharding: core i -> (batch b = i // 4, head-group g = i % 4). Each core
# computes attention for its batch's 4 heads (feature columns 256g:256g+256 of
# the QKV projections, rows 256g:256g+256 of Wo) and produces a partial
# out-projection [S, D] in bf16. Host sums the 4 partials per batch and adds
# the constant row bv @ Wo + bo (softmax rows sum to 1, so the V bias
# contributes a constant; bk drops entirely -- it shifts every score of a
# query equally, which softmax ignores).
#
# All host tensors are pre-tiled/pre-transposed to the [128, X] SBUF layout so
# every DMA is plain and contiguous (x ships as x^T bf16 -- no DMA transposes,
# no on-chip reconstruction).
#
# Per-core dataflow, bf16 operands everywhere (fp32 PSUM accumulation):
#   1. QT = Wq_s^T xT (+ scaled bq via DVE tensor_scalar) [256, S] and
#      KT [256, S], features on partitions, two heads per tile packed in
#      64-partition halves.
#   2. V = xT^T Wv_s [S, 256] (seq on partitions), stored per head augmented
#      with a ones column ([V_h | 1]) so the attention matmul also accumulates
#      the softmax denominator.
#   3. Scores per (head-pair t, key tile j): TWO CONCURRENT K=64 matmuls via
#      PE row tiling (head 2t in array rows 0:64, head 2t+1 in rows 64:128,
#      each streaming its own moving operand) into separate PSUM banks.
#      exp() on ScalarE (scores pre-scaled by 1/sqrt(Dh) via host-side Wq
#      scaling; magnitudes are small enough that max-subtraction is
#      unnecessary). Causality: skip k>q tiles + triangular mask multiply on
#      the diagonal block; [ctx^T; denom] += [V_h | 1]^T E, with the attnV
#      matmul deferred one j so the PE never stalls on the exp.
#   4. normalize: recip(denom) on DVE (via SBUF copy), partition-broadcast on
#      GPSIMD, scale ctx^T.
#   5. out_partial = ctxT^T Wo_s in bf16, deferred into the next chunk's
#      attention phase.

import numpy as np
import ml_dtypes

import concourse.bass as bass
import concourse.mybir as mybir
import concourse.tile as tile
from concourse import bacc
from concourse.bass_utils import run_bass_kernel_spmd
from concourse.masks import make_upper_triangular

F32 = mybir.dt.float32
BF16 = mybir.dt.bfloat16

B, S, D = 2, 2048, 1024
H, DH = 16, 64
NCORES = 8
GROUPS = 4               # head-groups (tensor parallel)
HG = H // GROUPS         # 4 heads per group
FEAT = HG * DH           # 256 features per group
SCALE = 1.0 / 8.0        # 1/sqrt(DH), folded into Wq/bq on host

CHUNK = 512              # seq chunk (PSUM bank = 512 fp32)
NSUB = CHUNK // 128      # 4 seq subtiles per chunk
NCHUNK = S // CHUNK      # 4
KD = D // 128            # 8 k-tiles over D
MT = FEAT // 128         # 2 feature M-tiles per group (= head pairs)
NT = S // 128            # 16 seq subtiles

DEBUG_TAPS = False


def _emit(tc):
    nc = tc.nc
    xt_d = nc.dram_tensor("xt", [128, KD * S], BF16, kind="ExternalInput").ap()
    wq_d = nc.dram_tensor("wq", [128, KD * FEAT], BF16,
                          kind="ExternalInput").ap()
    wk_d = nc.dram_tensor("wk", [128, KD * FEAT], BF16,
                          kind="ExternalInput").ap()
    wv_d = nc.dram_tensor("wv", [128, KD * FEAT], BF16,
                          kind="ExternalInput").ap()
    wo_d = nc.dram_tensor("wo", [128, MT * D], BF16, kind="ExternalInput").ap()
    bq_d = nc.dram_tensor("bq", [128, MT], F32, kind="ExternalInput").ap()
    out = nc.dram_tensor("out", [S, D], BF16, kind="ExternalOutput").ap()

    consts = tc.alloc_tile_pool(name="consts", bufs=1)
    weights = tc.alloc_tile_pool(name="weights", bufs=1)
    persist = tc.alloc_tile_pool(name="persist", bufs=1)
    qt_pool = tc.alloc_tile_pool(name="qt", bufs=2)
    et_pool = tc.alloc_tile_pool(name="et", bufs=6)
    rc_pool = tc.alloc_tile_pool(name="rc", bufs=2)
    ob_pool = tc.alloc_tile_pool(name="ob", bufs=2)
    sc_ps = tc.alloc_tile_pool(name="sc_ps", bufs=6, space="PSUM")  # 6 banks
    cx_ps = tc.alloc_tile_pool(name="cx_ps", bufs=2, space="PSUM")  # 2 banks

    tri = consts.tile([128, 128], BF16)
    make_upper_triangular(nc, tri, val=1.0, diag=True)

    # weights (pre-tiled on host; contiguous DMAs, split across queues)
    wq_sb = weights.tile([128, KD, MT, 128], BF16)
    wk_sb = weights.tile([128, KD, MT, 128], BF16)
    wv_sb = weights.tile([128, KD, FEAT], BF16)
    wo_sb = weights.tile([128, MT, D], BF16)
    bqt = weights.tile([128, MT], F32)
    nc.sync.dma_start(bqt, bq_d)
    for h in range(2):
        ks = slice(h * 4, h * 4 + 4)
        fs = slice(h * 4 * FEAT, (h + 1) * 4 * FEAT)
        nc.sync.dma_start(wq_sb[:, ks], wq_d[:, fs].rearrange(
            "p (k m f) -> p k m f", k=4, m=MT))
        nc.sync.dma_start(wk_sb[:, ks], wk_d[:, fs].rearrange(
            "p (k m f) -> p k m f", k=4, m=MT))
        nc.sync.dma_start(wv_sb[:, ks], wv_d[:, fs].rearrange(
            "p (k f) -> p k f", k=4))
        nc.sync.dma_start(wo_sb[:, h], wo_d[:, h * D:(h + 1) * D])

    # x^T bf16, fully resident; DMA split by (chunk, k-pair) so chunk 0's
    # slices land first and later chunks stream in behind the compute.
    xt = persist.tile([128, KD, S], BF16)
    for c in range(NCHUNK):
        for k in range(0, KD, 2):
            nc.sync.dma_start(
                xt[:, k:k + 2, c * CHUNK:(c + 1) * CHUNK],
                xt_d.rearrange("p (k s) -> p k s", k=KD)[
                    :, k:k + 2, c * CHUNK:(c + 1) * CHUNK])

    # persistent activations
    kt4 = persist.tile([128, MT, S], BF16)      # K^T; head 2t+i in rows 64i
    vaug = persist.tile([128, NT, HG, DH + 1], BF16)   # [V_h | 1] per head
    ctxT = persist.tile([128, MT, S], BF16)     # normalized ctx^T
    nc.vector.memset(vaug[:, :, :, DH], 1.0)

    def outproj(c):
        for t in range(NSUB):
            gt = c * NSUB + t
            ob = ob_pool.tile([128, D], BF16)
            for n in range(D // 512):
                op = sc_ps.tile([128, CHUNK], F32, tag="sc", name="op")
                for m in range(MT):
                    nc.tensor.matmul(
                        op,
                        ctxT[:, m, gt * 128:(gt + 1) * 128],
                        wo_sb[:, m, 512 * n:512 * (n + 1)],
                        start=(m == 0), stop=(m == MT - 1))
                nc.vector.tensor_copy(ob[:, 512 * n:512 * (n + 1)], op)
            nc.sync.dma_start(out[gt * 128:(gt + 1) * 128, :], ob)

    for c in range(NCHUNK):
        cs = c * CHUNK
        xc = xt[:, :, cs:cs + CHUNK]
        qt = qt_pool.tile([128, MT, CHUNK], BF16, name="qt")

        def proj_q():
            for m in range(MT):
                ps = sc_ps.tile([128, CHUNK], F32, tag="sc", name="psq")
                for k in range(KD):
                    nc.tensor.matmul(ps, wq_sb[:, k, m, :], xc[:, k, :],
                                     start=(k == 0), stop=(k == KD - 1))
                nc.vector.tensor_scalar_add(qt[:, m, :], ps, bqt[:, m:m + 1])

        def proj_k():
            for m in range(MT):
                ps = sc_ps.tile([128, CHUNK], F32, tag="sc", name="psk")
                for k in range(KD):
                    nc.tensor.matmul(ps, wk_sb[:, k, m, :], xc[:, k, :],
                                     start=(k == 0), stop=(k == KD - 1))
                for i in range(2):
                    r = slice(64 * i, 64 * i + 64)
                    nc.vector.tensor_copy(kt4[r, m, cs:cs + CHUNK], ps[r, :])

        def proj_v():
            for t in range(NSUB):
                gt = c * NSUB + t
                ps = sc_ps.tile([128, CHUNK], F32, tag="sc", name="psv")
                for k in range(KD):
                    nc.tensor.matmul(ps[:, 0:FEAT],
                                     xc[:, k, t * 128:(t + 1) * 128],
                                     wv_sb[:, k, :],
                                     start=(k == 0), stop=(k == KD - 1))
                nc.vector.tensor_copy(
                    vaug[:, gt, :, 0:DH],
                    ps[:, 0:FEAT].rearrange("p (h f) -> p h f", h=HG))

        def attn_j(t, cx2, j, jmax):
            """Row-tiled concurrent score pair + exp for key tile j of head
            pair (2t, 2t+1); returns the deferred attnV emitter."""
            lv = max(0, 128 * j - cs)
            nq = CHUNK - lv
            pp = [sc_ps.tile([128, CHUNK], F32, tag="sc", name="pp")
                  for _ in range(2)]
            for i in range(2):
                r = slice(64 * i, 64 * i + 64)
                nc.tensor.matmul(pp[i][:, 0:nq],
                                 kt4[r, t, 128 * j:128 * (j + 1)],
                                 qt[r, t, lv:CHUNK])
            ets = []
            diag = 128 * j >= cs
            for i in range(2):
                et = et_pool.tile([128, CHUNK], BF16, name="et")
                nc.scalar.activation(et[:, 0:nq], pp[i][:, 0:nq],
                                     mybir.ActivationFunctionType.Exp)
                if diag:
                    nc.vector.tensor_mul(et[:, 0:128], et[:, 0:128], tri)
                ets.append(et)

            def emit_av():
                for i in range(2):
                    nc.tensor.matmul(cx2[i][:, lv:CHUNK],
                                     vaug[:, j, 2 * t + i, :],
                                     ets[i][:, 0:nq],
                                     start=(j == 0), stop=(j == jmax - 1),
                                     skip_group_check=True)
            return emit_av

        def normalize(t, cx2):
            for i in range(2):
                rc0 = rc_pool.tile([1, CHUNK], F32, tag="rc0")
                nc.vector.tensor_copy(rc0, cx2[i][DH:DH + 1, :])
                rc = rc_pool.tile([1, CHUNK], F32, tag="rc")
                nc.vector.reciprocal_approx_fast(rc, rc0)
                bcs = rc_pool.tile([64, CHUNK], F32, tag="bcs")
                nc.gpsimd.partition_broadcast(bcs, rc)
                nc.vector.tensor_mul(ctxT[64 * i:64 * i + 64, t, cs:cs + CHUNK],
                                     cx2[i][0:DH, :], bcs)

        proj_q()
        proj_k()
        proj_v()
        jmax = (c + 1) * NSUB
        for t in range(MT):
            cx2 = [cx_ps.tile([DH + 1, CHUNK], F32, tag="cx", name="cx")
                   for _ in range(2)]
            pend = None
            for j in range(jmax):
                nxt = attn_j(t, cx2, j, jmax)
                if pend is not None:
                    pend()
                pend = nxt
                if j == 1 and t == 0 and c > 0:
                    outproj(c - 1)
            pend()
            normalize(t, cx2)
    outproj(NCHUNK - 1)

    if DEBUG_TAPS:
        kt_dbg = nc.dram_tensor("kt_dbg", [128, MT, S], BF16,
                                kind="ExternalOutput").ap()
        nc.sync.dma_start(kt_dbg, kt4)
        va_dbg = nc.dram_tensor("va_dbg", [128, NT, HG, DH + 1], BF16,
                                kind="ExternalOutput").ap()
        nc.sync.dma_start(va_dbg, vaug)
        cx_dbg = nc.dram_tensor("cx_dbg", [128, MT, S], BF16,
                                kind="ExternalOutput").ap()
        nc.sync.dma_start(cx_dbg, ctxT)

    for p in (cx_ps, sc_ps, ob_pool, rc_pool, et_pool, qt_pool,
              persist, weights, consts):
        p.release()


_BUILT = None


def _build():
    global _BUILT
    if _BUILT is None:
        nc = bacc.Bacc("TRN2", target_bir_lowering=False, debug=False,
                       num_devices=NCORES)
        with tile.TileContext(nc) as tc:
            _emit(tc)
        nc.compile()
        _BUILT = nc
    return _BUILT


def _b16(a):
    return np.ascontiguousarray(np.asarray(a, np.float32)).astype(
        ml_dtypes.bfloat16)


def _shards(inputs):
    x = np.asarray(inputs["x"], np.float32)
    Wq = np.asarray(inputs["Wq"], np.float32)
    Wk = np.asarray(inputs["Wk"], np.float32)
    Wv = np.asarray(inputs["Wv"], np.float32)
    Wo = np.asarray(inputs["Wo"], np.float32)
    bq = np.asarray(inputs["bq"], np.float32)

    def ktile(w):  # [D, X] -> [128, KD*X]
        return w.reshape(KD, 128, -1).transpose(1, 0, 2).reshape(128, -1)

    maps = []
    for core in range(NCORES):
        b, g = core // GROUPS, core % GROUPS
        fs = slice(g * FEAT, (g + 1) * FEAT)
        maps.append({
            "xt": _b16(ktile(x[b].T)),
            "wq": _b16(ktile(Wq[:, fs] * SCALE)),
            "wk": _b16(ktile(Wk[:, fs])),
            "wv": _b16(ktile(Wv[:, fs])),
            "wo": _b16(Wo[fs, :].reshape(MT, 128, D).transpose(
                1, 0, 2).reshape(128, -1)),
            "bq": np.ascontiguousarray((bq[fs] * SCALE).reshape(MT, 128).T),
        })
    return maps


def kernel(trace=False, **inputs):
    nc = _build()
    res = run_bass_kernel_spmd(nc, _shards(inputs), core_ids=list(range(NCORES)),
                               trace=trace)
    partial = np.stack([r_["out"] for r_ in res.results])  # [8, S, D] bf16
    acc = partial.astype(np.float64).reshape(B, GROUPS, S, D).sum(axis=1)
    acc += (np.asarray(inputs["bv"], np.float64) @
            np.asarray(inputs["Wo"], np.float64) +
            np.asarray(inputs["bo"], np.float64))
    out = acc.astype(np.float32)
    if trace:
        return out, res
    return out


# revision 9
# speedup vs baseline: 1.0003x; 1.0003x over previous
# Multi-head causal self-attention (B=2, S=2048, D=1024, H=16, Dh=64) on 8
# Trainium2 NeuronCores.
#
# Sharding: core i -> (batch b = i // 4, head-group g = i % 4). Each core
# computes attention for its batch's 4 heads (feature columns 256g:256g+256 of
# the QKV projections, rows 256g:256g+256 of Wo) and produces a partial
# out-projection [S, D] in bf16. Host sums the 4 partials per batch and adds
# the constant row bv @ Wo + bo (softmax rows sum to 1, so the V bias
# contributes a constant; bk drops entirely -- it shifts every score of a
# query equally, which softmax ignores).
#
# All host tensors are pre-tiled/pre-transposed to the [128, X] SBUF layout so
# every DMA is plain and contiguous (x ships as x^T bf16 -- no DMA transposes,
# no on-chip reconstruction).
#
# Per-core dataflow, bf16 operands everywhere (fp32 PSUM accumulation):
#   1. QT = Wq_s^T xT (+ scaled bq via DVE tensor_scalar) [256, S] and
#      KT [256, S], features on partitions, two heads per tile packed in
#      64-partition halves.
#   2. V = xT^T Wv_s [S, 256] (seq on partitions), stored per head augmented
#      with a ones column ([V_h | 1]) so the attention matmul also accumulates
#      the softmax denominator.
#   3. Scores per (head-pair t, key tile j): TWO CONCURRENT K=64 matmuls via
#      PE row tiling (head 2t in array rows 0:64, head 2t+1 in rows 64:128,
#      each streaming its own moving operand) into separate PSUM banks.
#      exp() on ScalarE (scores pre-scaled by 1/sqrt(Dh) via host-side Wq
#      scaling; magnitudes are small enough that max-subtraction is
#      unnecessary). Causality: skip k>q tiles + triangular mask multiply on
#      the diagonal block; [ctx^T; denom] += [V_h | 1]^T E, with the attnV
#      matmul deferred one j so the PE never stalls on the exp.
#   4. normalize: recip(denom) on DVE (via SBUF copy), partition-broadcast on
#      GPSIMD, scale ctx^T.
#   5. out_partial = ctxT^T Wo_s in bf16, deferred into the next chunk's
#      attention phase.

import numpy as np
import ml_dtypes

import concourse.bass as bass
import concourse.mybir as mybir
import concourse.tile as tile
from concourse import bacc
from concourse.bass_utils import run_bass_kernel_spmd
from concourse.masks import make_upper_triangular

F32 = mybir.dt.float32
BF16 = mybir.dt.bfloat16

B, S, D = 2, 2048, 1024
H, DH = 16, 64
NCORES = 8
GROUPS = 4               # head-groups (tensor parallel)
HG = H // GROUPS         # 4 heads per group
FEAT = HG * DH           # 256 features per group
SCALE = 1.0 / 8.0        # 1/sqrt(DH), folded into Wq/bq on host

CHUNK = 512              # seq chunk (PSUM bank = 512 fp32)
NSUB = CHUNK // 128      # 4 seq subtiles per chunk
NCHUNK = S // CHUNK      # 4
KD = D // 128            # 8 k-tiles over D
MT = FEAT // 128         # 2 feature M-tiles per group (= head pairs)
NT = S // 128            # 16 seq subtiles

DEBUG_TAPS = False


def _emit(tc):
    nc = tc.nc
    xt_d = nc.dram_tensor("xt", [128, KD * S], BF16, kind="ExternalInput").ap()
    wq_d = nc.dram_tensor("wq", [128, KD * FEAT], BF16,
                          kind="ExternalInput").ap()
    wk_d = nc.dram_tensor("wk", [128, KD * FEAT], BF16,
                          kind="ExternalInput").ap()
    wv_d = nc.dram_tensor("wv", [128, KD * FEAT], BF16,
                          kind="ExternalInput").ap()
    wo_d = nc.dram_tensor("wo", [128, MT * D], BF16, kind="ExternalInput").ap()
    bq_d = nc.dram_tensor("bq", [128, MT], F32, kind="ExternalInput").ap()
    out = nc.dram_tensor("out", [S, D], BF16, kind="ExternalOutput").ap()

    consts = tc.alloc_tile_pool(name="consts", bufs=1)
    weights = tc.alloc_tile_pool(name="weights", bufs=1)
    persist = tc.alloc_tile_pool(name="persist", bufs=1)
    qt_pool = tc.alloc_tile_pool(name="qt", bufs=2)
    et_pool = tc.alloc_tile_pool(name="et", bufs=6)
    rc_pool = tc.alloc_tile_pool(name="rc", bufs=2)
    ob_pool = tc.alloc_tile_pool(name="ob", bufs=2)
    sc_ps = tc.alloc_tile_pool(name="sc_ps", bufs=6, space="PSUM")  # 6 banks
    cx_ps = tc.alloc_tile_pool(name="cx_ps", bufs=2, space="PSUM")  # 2 banks

    tri = consts.tile([128, 128], BF16)
    make_upper_triangular(nc, tri, val=1.0, diag=True)

    # weights (pre-tiled on host; contiguous DMAs, split across queues)
    wq_sb = weights.tile([128, KD, MT, 128], BF16)
    wk_sb = weights.tile([128, KD, MT, 128], BF16)
    wv_sb = weights.tile([128, KD, FEAT], BF16)
    wo_sb = weights.tile([128, MT, D], BF16)
    bqt = weights.tile([128, MT], F32)
    nc.sync.dma_start(bqt, bq_d)
    for h in range(2):
        ks = slice(h * 4, h * 4 + 4)
        fs = slice(h * 4 * FEAT, (h + 1) * 4 * FEAT)
        nc.sync.dma_start(wq_sb[:, ks], wq_d[:, fs].rearrange(
            "p (k m f) -> p k m f", k=4, m=MT))
        nc.sync.dma_start(wk_sb[:, ks], wk_d[:, fs].rearrange(
            "p (k m f) -> p k m f", k=4, m=MT))
        nc.sync.dma_start(wv_sb[:, ks], wv_d[:, fs].rearrange(
            "p (k f) -> p k f", k=4))
        nc.sync.dma_start(wo_sb[:, h], wo_d[:, h * D:(h + 1) * D])

    # x^T bf16, fully resident; DMA split by (chunk, k-pair) so chunk 0's
    # slices land first and later chunks stream in behind the compute.
    xt = persist.tile([128, KD, S], BF16)
    for c in range(NCHUNK):
        for k in range(0, KD, 2):
            nc.sync.dma_start(
                xt[:, k:k + 2, c * CHUNK:(c + 1) * CHUNK],
                xt_d.rearrange("p (k s) -> p k s", k=KD)[
                    :, k:k + 2, c * CHUNK:(c + 1) * CHUNK])

    # persistent activations
    kt4 = persist.tile([128, MT, S], BF16)      # K^T; head 2t+i in rows 64i
    vaug = persist.tile([128, NT, HG, DH + 1], BF16)   # [V_h | 1] per head
    ctxT = persist.tile([128, MT, S], BF16)     # normalized ctx^T
    nc.vector.memset(vaug[:, :, :, DH], 1.0)

    def outproj(c):
        for t in range(NSUB):
            gt = c * NSUB + t
            ob = ob_pool.tile([128, D], BF16)
            for n in range(D // 512):
                op = sc_ps.tile([128, CHUNK], F32, tag="sc", name="op")
                for m in range(MT):
                    nc.tensor.matmul(
                        op,
                        ctxT[:, m, gt * 128:(gt + 1) * 128],
                        wo_sb[:, m, 512 * n:512 * (n + 1)],
                        start=(m == 0), stop=(m == MT - 1))
                nc.vector.tensor_copy(ob[:, 512 * n:512 * (n + 1)], op)
            nc.sync.dma_start(out[gt * 128:(gt + 1) * 128, :], ob)

    def proj_emitters(c, qt):
        """8 closures covering chunk c's Q/K/V projections, to be interleaved
        into the previous chunk's (ACT-bound) attention phase."""
        cs = c * CHUNK
        xc = xt[:, :, cs:cs + CHUNK]
        ems = []

        def q_m(m):
            def em():
                ps = sc_ps.tile([128, CHUNK], F32, tag="sc", name="psq")
                for k in range(KD):
                    nc.tensor.matmul(ps, wq_sb[:, k, m, :], xc[:, k, :],
                                     start=(k == 0), stop=(k == KD - 1))
                nc.vector.tensor_scalar_add(qt[:, m, :], ps, bqt[:, m:m + 1])
            return em

        def k_m(m):
            def em():
                ps = sc_ps.tile([128, CHUNK], F32, tag="sc", name="psk")
                for k in range(KD):
                    nc.tensor.matmul(ps, wk_sb[:, k, m, :], xc[:, k, :],
                                     start=(k == 0), stop=(k == KD - 1))
                for i in range(2):
                    r = slice(64 * i, 64 * i + 64)
                    nc.vector.tensor_copy(kt4[r, m, cs:cs + CHUNK], ps[r, :])
            return em

        def v_t(t):
            def em():
                gt = c * NSUB + t
                ps = sc_ps.tile([128, CHUNK], F32, tag="sc", name="psv")
                for k in range(KD):
                    nc.tensor.matmul(ps[:, 0:FEAT],
                                     xc[:, k, t * 128:(t + 1) * 128],
                                     wv_sb[:, k, :],
                                     start=(k == 0), stop=(k == KD - 1))
                nc.vector.tensor_copy(
                    vaug[:, gt, :, 0:DH],
                    ps[:, 0:FEAT].rearrange("p (h f) -> p h f", h=HG))
            return em

        for m in range(MT):
            ems.append(q_m(m))
        for m in range(MT):
            ems.append(k_m(m))
        for t in range(NSUB):
            ems.append(v_t(t))
        return ems

    qt_cur = qt_pool.tile([128, MT, CHUNK], BF16, name="qt_cur")
    for em in proj_emitters(0, qt_cur):
        em()

    for c in range(NCHUNK):
        cs = c * CHUNK
        qt = qt_cur

        def attn_j(t, cx2, j, jmax):
            """Row-tiled concurrent score pair + exp for key tile j of head
            pair (2t, 2t+1); returns the deferred attnV emitter."""
            lv = max(0, 128 * j - cs)
            nq = CHUNK - lv
            pp = [sc_ps.tile([128, CHUNK], F32, tag="sc", name="pp")
                  for _ in range(2)]
            for i in range(2):
                r = slice(64 * i, 64 * i + 64)
                nc.tensor.matmul(pp[i][:, 0:nq],
                                 kt4[r, t, 128 * j:128 * (j + 1)],
                                 qt[r, t, lv:CHUNK])
            ets = []
            diag = 128 * j >= cs
            for i in range(2):
                et = et_pool.tile([128, CHUNK], BF16, name="et")
                nc.scalar.activation(et[:, 0:nq], pp[i][:, 0:nq],
                                     mybir.ActivationFunctionType.Exp)
                if diag:
                    nc.vector.tensor_mul(et[:, 0:128], et[:, 0:128], tri)
                ets.append(et)

            def emit_av():
                for i in range(2):
                    nc.tensor.matmul(cx2[i][:, lv:CHUNK],
                                     vaug[:, j, 2 * t + i, :],
                                     ets[i][:, 0:nq],
                                     start=(j == 0), stop=(j == jmax - 1),
                                     skip_group_check=True)
            return emit_av

        def normalize(t, cx2):
            for i in range(2):
                rc0 = rc_pool.tile([1, CHUNK], F32, tag="rc0")
                nc.vector.tensor_copy(rc0, cx2[i][DH:DH + 1, :])
                rc = rc_pool.tile([1, CHUNK], F32, tag="rc")
                nc.vector.reciprocal_approx_fast(rc, rc0)
                bcs = rc_pool.tile([64, CHUNK], F32, tag="bcs")
                nc.gpsimd.partition_broadcast(bcs, rc)
                nc.vector.tensor_mul(ctxT[64 * i:64 * i + 64, t, cs:cs + CHUNK],
                                     cx2[i][0:DH, :], bcs)

        if c + 1 < NCHUNK:
            qt_nxt = qt_pool.tile([128, MT, CHUNK], BF16, name="qt_nxt")
            pending_proj = proj_emitters(c + 1, qt_nxt)
        else:
            qt_nxt = None
            pending_proj = []
        jmax = (c + 1) * NSUB
        # spread next-chunk projections over t=1's j loop, previous chunk's
        # out-projection over t=0's
        for t in range(MT):
            cx2 = [cx_ps.tile([DH + 1, CHUNK], F32, tag="cx", name="cx")
                   for _ in range(2)]
            pend = None
            for j in range(jmax):
                if pend is not None:
                    pend()
                pend = attn_j(t, cx2, j, jmax)
                if t == 0 and c > 0 and j == 1:
                    outproj(c - 1)
                if t == 1:
                    for _ in range((len(pending_proj) + jmax - j - 1) //
                                   max(1, jmax - j)):
                        pending_proj.pop(0)()
            pend()
            normalize(t, cx2)
        qt_cur = qt_nxt
    outproj(NCHUNK - 1)

    if DEBUG_TAPS:
        kt_dbg = nc.dram_tensor("kt_dbg", [128, MT, S], BF16,
                                kind="ExternalOutput").ap()
        nc.sync.dma_start(kt_dbg, kt4)
        va_dbg = nc.dram_tensor("va_dbg", [128, NT, HG, DH + 1], BF16,
                                kind="ExternalOutput").ap()
        nc.sync.dma_start(va_dbg, vaug)
        cx_dbg = nc.dram_tensor("cx_dbg", [128, MT, S], BF16,
                                kind="ExternalOutput").ap()
        nc.sync.dma_start(cx_dbg, ctxT)

    for p in (cx_ps, sc_ps, ob_pool, rc_pool, et_pool, qt_pool,
              persist, weights, consts):
        p.release()


_BUILT = None


def _build():
    global _BUILT
    if _BUILT is None:
        nc = bacc.Bacc("TRN2", target_bir_lowering=False, debug=False,
                       num_devices=NCORES)
        with tile.TileContext(nc) as tc:
            _emit(tc)
        nc.compile()
        _BUILT = nc
    return _BUILT


def _b16(a):
    return np.ascontiguousarray(np.asarray(a, np.float32)).astype(
        ml_dtypes.bfloat16)


def _shards(inputs):
    x = np.asarray(inputs["x"], np.float32)
    Wq = np.asarray(inputs["Wq"], np.float32)
    Wk = np.asarray(inputs["Wk"], np.float32)
    Wv = np.asarray(inputs["Wv"], np.float32)
    Wo = np.asarray(inputs["Wo"], np.float32)
    bq = np.asarray(inputs["bq"], np.float32)

    def ktile(w):  # [D, X] -> [128, KD*X]
        return w.reshape(KD, 128, -1).transpose(1, 0, 2).reshape(128, -1)

    maps = []
    for core in range(NCORES):
        b, g = core // GROUPS, core % GROUPS
        fs = slice(g * FEAT, (g + 1) * FEAT)
        maps.append({
            "xt": _b16(ktile(x[b].T)),
            "wq": _b16(ktile(Wq[:, fs] * SCALE)),
            "wk": _b16(ktile(Wk[:, fs])),
            "wv": _b16(ktile(Wv[:, fs])),
            "wo": _b16(Wo[fs, :].reshape(MT, 128, D).transpose(
                1, 0, 2).reshape(128, -1)),
            "bq": np.ascontiguousarray((bq[fs] * SCALE).reshape(MT, 128).T),
        })
    return maps


def kernel(trace=False, **inputs):
    nc = _build()
    res = run_bass_kernel_spmd(nc, _shards(inputs), core_ids=list(range(NCORES)),
                               trace=trace)
    partial = np.stack([r_["out"] for r_ in res.results])  # [8, S, D] bf16
    acc = partial.astype(np.float64).reshape(B, GROUPS, S, D).sum(axis=1)
    acc += (np.asarray(inputs["bv"], np.float64) @
            np.asarray(inputs["Wo"], np.float64) +
            np.asarray(inputs["bo"], np.float64))
    out = acc.astype(np.float32)
    if trace:
        return out, res
    return out


# revision 13
# speedup vs baseline: 1.0236x; 1.0233x over previous
# Multi-head causal self-attention (B=2, S=2048, D=1024, H=16, Dh=64) on 8
# Trainium2 NeuronCores.
#
# Sharding: core i -> (batch b = i // 4, head-group g = i % 4). Each core
# computes attention for its batch's 4 heads (feature columns 256g:256g+256 of
# the QKV projections, rows 256g:256g+256 of Wo) and produces a partial
# out-projection [S, D] in bf16. Host sums the 4 partials per batch and adds
# the constant row bv @ Wo + bo (softmax rows sum to 1, so the V bias
# contributes a constant; bk drops entirely -- it shifts every score of a
# query equally, which softmax ignores).
#
# All host tensors are pre-tiled/pre-transposed to the [128, X] SBUF layout so
# every DMA is plain and contiguous (x ships as x^T bf16 -- no DMA transposes).
# DMA issue is spread across the sync/gpsimd/scalar queues (descriptor issue
# costs ~0.6us on the issuing engine's queue).
#
# Per-core dataflow, bf16 operands everywhere (fp32 PSUM accumulation):
#   1. QT = Wq_s^T xT (+ scaled bq via DVE tensor_scalar) [256, S] and
#      KT [256, S], features on partitions, two heads per tile packed in
#      64-partition halves.
#   2. V = xT^T Wv_s [S, 256] (seq on partitions), stored per head augmented
#      with a ones column ([V_h | 1]) so the attention matmul also accumulates
#      the softmax denominator.
#   3. Scores per (head-pair t, key tile j): TWO CONCURRENT K=64 matmuls via
#      PE row tiling (head 2t in array rows 0:64, head 2t+1 in rows 64:128,
#      each streaming its own moving operand) into the two banks of ONE
#      [128, 2, 512] PSUM tile, consumed by ONE exp() on ScalarE. The single
#      producer/consumer keeps the score pair's dependencies symmetric so the
#      Tile scheduler leaves the pair adjacent (adjacency is what makes the
#      row-tiled matmuls actually overlap on the PE array). Causality: skip
#      k>q tiles + triangular mask multiply on the diagonal block;
#      [ctx^T; denom](h) += [V_h | 1]^T E, deferred one j so the PE never
#      stalls on the exp.
#   4. normalize: recip(denom) on DVE (via SBUF copy), partition-broadcast on
#      GPSIMD, scale ctx^T.
#   5. out_partial = ctxT^T Wo_s in bf16. Q/K/V projections of chunk c+1 and
#      the out-projection of chunk c-1 are interleaved into chunk c's
#      attention phase to keep the PE dense (HAM clock-gate stays warm).

import numpy as np
import ml_dtypes

import concourse.bass as bass
import concourse.mybir as mybir
import concourse.tile as tile
from concourse import bacc
from concourse.bass_utils import run_bass_kernel_spmd
from concourse.masks import make_upper_triangular

F32 = mybir.dt.float32
BF16 = mybir.dt.bfloat16

B, S, D = 2, 2048, 1024
H, DH = 16, 64
NCORES = 8
GROUPS = 4               # head-groups (tensor parallel)
HG = H // GROUPS         # 4 heads per group
FEAT = HG * DH           # 256 features per group
SCALE = 1.0 / 8.0        # 1/sqrt(DH), folded into Wq/bq on host

CHUNK = 512              # seq chunk (PSUM bank = 512 fp32)
NSUB = CHUNK // 128      # 4 seq subtiles per chunk
NCHUNK = S // CHUNK      # 4
KD = D // 128            # 8 k-tiles over D
MT = FEAT // 128         # 2 feature M-tiles per group (= head pairs)
NT = S // 128            # 16 seq subtiles

DEBUG_TAPS = False


def _emit(tc):
    nc = tc.nc
    xt_d = nc.dram_tensor("xt", [128, KD * S], BF16, kind="ExternalInput").ap()
    wq_d = nc.dram_tensor("wq", [128, KD * FEAT], BF16,
                          kind="ExternalInput").ap()
    wk_d = nc.dram_tensor("wk", [128, KD * FEAT], BF16,
                          kind="ExternalInput").ap()
    wv_d = nc.dram_tensor("wv", [128, KD * FEAT], BF16,
                          kind="ExternalInput").ap()
    wo_d = nc.dram_tensor("wo", [128, MT * D], BF16, kind="ExternalInput").ap()
    bq_d = nc.dram_tensor("bq", [128, MT], F32, kind="ExternalInput").ap()
    out = nc.dram_tensor("out", [S, D], BF16, kind="ExternalOutput").ap()

    consts = tc.alloc_tile_pool(name="consts", bufs=1)
    weights = tc.alloc_tile_pool(name="weights", bufs=1)
    persist = tc.alloc_tile_pool(name="persist", bufs=1)
    qt_pool = tc.alloc_tile_pool(name="qt", bufs=2)
    et_pool = tc.alloc_tile_pool(name="et", bufs=4)
    rc_pool = tc.alloc_tile_pool(name="rc", bufs=2)
    ob_pool = tc.alloc_tile_pool(name="ob", bufs=2)
    work_ps = tc.alloc_tile_pool(name="work_ps", bufs=1, space="PSUM")

    # DMA issue engines, round-robin (sync + the mostly-idle gpsimd queue;
    # scalar/vector queues are not allowed or too busy)
    dma_eng = [nc.sync, nc.gpsimd]
    dma_rr = [0]

    def dma(dst, src):
        e = dma_eng[dma_rr[0] % len(dma_eng)]
        dma_rr[0] += 1
        e.dma_start(dst, src)

    tri = consts.tile([128, 128], BF16)
    make_upper_triangular(nc, tri, val=1.0, diag=True)

    # weights (pre-tiled on host; contiguous DMAs, split across queues)
    wq_sb = weights.tile([128, KD, MT, 128], BF16)
    wk_sb = weights.tile([128, KD, MT, 128], BF16)
    wv_sb = weights.tile([128, KD, FEAT], BF16)
    wo_sb = weights.tile([128, MT, D], BF16)
    bqt = weights.tile([128, MT], F32)
    xt = persist.tile([128, KD, S], BF16)

    def dma_in():
        # first-needed first: wq + x chunk 0, then the rest
        for h in range(2):
            ks = slice(h * 4, h * 4 + 4)
            fs = slice(h * 4 * FEAT, (h + 1) * 4 * FEAT)
            dma(wq_sb[:, ks], wq_d[:, fs].rearrange(
                "p (k m f) -> p k m f", k=4, m=MT))
        xv = xt_d.rearrange("p (k s) -> p k s", k=KD)
        for c in range(NCHUNK):
            for k in range(0, KD, 4):
                dma(xt[:, k:k + 4, c * CHUNK:(c + 1) * CHUNK],
                    xv[:, k:k + 4, c * CHUNK:(c + 1) * CHUNK])
            if c == 0:
                for h in range(2):
                    ks = slice(h * 4, h * 4 + 4)
                    fs = slice(h * 4 * FEAT, (h + 1) * 4 * FEAT)
                    dma(wk_sb[:, ks], wk_d[:, fs].rearrange(
                        "p (k m f) -> p k m f", k=4, m=MT))
                    dma(wv_sb[:, ks], wv_d[:, fs].rearrange(
                        "p (k f) -> p k f", k=4))
                dma(bqt, bq_d)
            if c == 1:
                for h in range(2):
                    dma(wo_sb[:, h], wo_d[:, h * D:(h + 1) * D])

    dma_in()

    # persistent activations
    kt4 = persist.tile([128, MT, S], BF16)      # K^T; head 2t+i in rows 64i
    vaug = persist.tile([128, NT, HG, DH + 1], BF16)   # [V_h | 1] per head
    ctxT = persist.tile([128, MT, S], BF16)     # normalized ctx^T
    nc.vector.memset(vaug[:, :, :, DH], 1.0)

    def outproj_emitters(c):
        ems = []
        obs = {}

        def gtn(t, n):
            def em():
                gt = c * NSUB + t
                if n == 0:
                    obs[t] = ob_pool.tile([128, D], BF16, name="ob")
                ob = obs[t]
                op = work_ps.tile([128, CHUNK], F32, tag="pj", bufs=1,
                                  name="op")
                for m in range(MT):
                    nc.tensor.matmul(
                        op,
                        ctxT[:, m, gt * 128:(gt + 1) * 128],
                        wo_sb[:, m, 512 * n:512 * (n + 1)],
                        start=(m == 0), stop=(m == MT - 1))
                nc.vector.tensor_copy(ob[:, 512 * n:512 * (n + 1)], op)
                if n == 1:
                    for hf in range(2):
                        dma(out[gt * 128 + 64 * hf:gt * 128 + 64 * (hf + 1), :],
                            ob[64 * hf:64 * (hf + 1), :])
            return em
        for t in range(NSUB):
            for n in range(D // 512):
                ems.append(gtn(t, n))
        return ems

    def proj_emitters(c, qt):
        """8 closures covering chunk c's Q/K/V projections, interleaved into
        the previous chunk's attention phase."""
        cs = c * CHUNK
        xc = xt[:, :, cs:cs + CHUNK]
        ems = []

        def q_m(m):
            def em():
                ps = work_ps.tile([128, CHUNK], F32, tag="pj", bufs=1,
                                  name="psq")
                for k in range(KD):
                    nc.tensor.matmul(ps, wq_sb[:, k, m, :], xc[:, k, :],
                                     start=(k == 0), stop=(k == KD - 1))
                nc.vector.tensor_scalar_add(qt[:, m, :], ps, bqt[:, m:m + 1])
            return em

        def k_m(m):
            def em():
                ps = work_ps.tile([128, CHUNK], F32, tag="pj", bufs=1,
                                  name="psk")
                for k in range(KD):
                    nc.tensor.matmul(ps, wk_sb[:, k, m, :], xc[:, k, :],
                                     start=(k == 0), stop=(k == KD - 1))
                for i in range(2):
                    r = slice(64 * i, 64 * i + 64)
                    nc.vector.tensor_copy(kt4[r, m, cs:cs + CHUNK], ps[r, :])
            return em

        def v_t(t):
            def em():
                gt = c * NSUB + t
                ps = work_ps.tile([128, CHUNK], F32, tag="pj", bufs=1,
                                  name="psv")
                for k in range(KD):
                    nc.tensor.matmul(ps[:, 0:FEAT],
                                     xc[:, k, t * 128:(t + 1) * 128],
                                     wv_sb[:, k, :],
                                     start=(k == 0), stop=(k == KD - 1))
                nc.vector.tensor_copy(
                    vaug[:, gt, :, 0:DH],
                    ps[:, 0:FEAT].rearrange("p (h f) -> p h f", h=HG))
            return em

        for m in range(MT):
            ems.append(q_m(m))
        for m in range(MT):
            ems.append(k_m(m))
        for t in range(NSUB):
            ems.append(v_t(t))
        return ems

    qt_cur = qt_pool.tile([128, MT, CHUNK], BF16, name="qt_cur")
    for em in proj_emitters(0, qt_cur):
        em()

    for c in range(NCHUNK):
        cs = c * CHUNK
        qt = qt_cur

        def attn_j(t, cx2, j, jmax):
            """Adjacent row-tiled score pair into one 2-bank PSUM tile, one
            exp for both heads; returns the deferred attnV emitter."""
            lv = max(0, 128 * j - cs)
            nq = CHUNK - lv
            pp = work_ps.tile([128, 2, CHUNK], F32, tag="sc2", bufs=2,
                              name="pp")
            for i in range(2):
                r = slice(64 * i, 64 * i + 64)
                nc.tensor.matmul(pp[:, i, 0:nq],
                                 kt4[r, t, 128 * j:128 * (j + 1)],
                                 qt[r, t, lv:CHUNK])
            et2 = et_pool.tile([128, 2, CHUNK], BF16, name="et2")
            nc.scalar.activation(et2[:, :, 0:nq], pp[:, :, 0:nq],
                                 mybir.ActivationFunctionType.Exp)
            if 128 * j >= cs:  # diagonal tile: causal mask
                for i in range(2):
                    nc.vector.tensor_mul(et2[:, i, 0:128], et2[:, i, 0:128],
                                         tri)

            def emit_av():
                for i in range(2):
                    nc.tensor.matmul(cx2[i][:, lv:CHUNK],
                                     vaug[:, j, 2 * t + i, :],
                                     et2[:, i, 0:nq],
                                     start=(j == 0), stop=(j == jmax - 1),
                                     skip_group_check=True)
            return emit_av

        def normalize(t, cx2):
            for i in range(2):
                rc0 = rc_pool.tile([1, CHUNK], F32, tag="rc0")
                nc.vector.tensor_copy(rc0, cx2[i][DH:DH + 1, :])
                rc = rc_pool.tile([1, CHUNK], F32, tag="rc")
                nc.vector.reciprocal_approx_fast(rc, rc0)
                bcs = rc_pool.tile([64, CHUNK], F32, tag="bcs")
                nc.gpsimd.partition_broadcast(bcs, rc)
                nc.vector.tensor_mul(ctxT[64 * i:64 * i + 64, t, cs:cs + CHUNK],
                                     cx2[i][0:DH, :], bcs)

        if c + 1 < NCHUNK:
            qt_nxt = qt_pool.tile([128, MT, CHUNK], BF16, name="qt_nxt")
            pending_proj = proj_emitters(c + 1, qt_nxt)
        else:
            qt_nxt = None
            pending_proj = []
        pending_out = outproj_emitters(c - 1) if c > 0 else []
        jmax = (c + 1) * NSUB
        # spread next-chunk projections over t=1's j loop, previous chunk's
        # out-projection over t=0's
        for t in range(MT):
            cx2 = [cx_tile for cx_tile in
                   (work_ps.tile([DH + 1, CHUNK], F32, tag="cx", bufs=3,
                                 name="cxa"),
                    work_ps.tile([DH + 1, CHUNK], F32, tag="cx", bufs=3,
                                 name="cxb"))]
            pend = None
            for j in range(jmax):
                if pend is not None:
                    pend()
                pend = attn_j(t, cx2, j, jmax)
                work = pending_out if t == 0 else pending_proj
                for _ in range((len(work) + jmax - j - 1) // max(1, jmax - j)):
                    work.pop(0)()
            pend()
            normalize(t, cx2)
        for em in pending_out + pending_proj:
            em()
        qt_cur = qt_nxt
    for em in outproj_emitters(NCHUNK - 1):
        em()

    if DEBUG_TAPS:
        kt_dbg = nc.dram_tensor("kt_dbg", [128, MT, S], BF16,
                                kind="ExternalOutput").ap()
        nc.sync.dma_start(kt_dbg, kt4)
        va_dbg = nc.dram_tensor("va_dbg", [128, NT, HG, DH + 1], BF16,
                                kind="ExternalOutput").ap()
        nc.sync.dma_start(va_dbg, vaug)
        cx_dbg = nc.dram_tensor("cx_dbg", [128, MT, S], BF16,
                                kind="ExternalOutput").ap()
        nc.sync.dma_start(cx_dbg, ctxT)

    for p in (work_ps, ob_pool, rc_pool, et_pool, qt_pool,
              persist, weights, consts):
        p.release()


_BUILT = None


def _build():
    global _BUILT
    if _BUILT is None:
        nc = bacc.Bacc("TRN2", target_bir_lowering=False, debug=False,
                       num_devices=NCORES)
        with tile.TileContext(nc) as tc:
            _emit(tc)
        nc.compile()
        _BUILT = nc
    return _BUILT


def _b16(a):
    return np.ascontiguousarray(np.asarray(a, np.float32)).astype(
        ml_dtypes.bfloat16)


def _shards(inputs):
    x = np.asarray(inputs["x"], np.float32)
    Wq = np.asarray(inputs["Wq"], np.float32)
    Wk = np.asarray(inputs["Wk"], np.float32)
    Wv = np.asarray(inputs["Wv"], np.float32)
    Wo = np.asarray(inputs["Wo"], np.float32)
    bq = np.asarray(inputs["bq"], np.float32)

    def ktile(w):  # [D, X] -> [128, KD*X]
        return w.reshape(KD, 128, -1).transpose(1, 0, 2).reshape(128, -1)

    maps = []
    for core in range(NCORES):
        b, g = core // GROUPS, core % GROUPS
        fs = slice(g * FEAT, (g + 1) * FEAT)
        maps.append({
            "xt": _b16(ktile(x[b].T)),
            "wq": _b16(ktile(Wq[:, fs] * SCALE)),
            "wk": _b16(ktile(Wk[:, fs])),
            "wv": _b16(ktile(Wv[:, fs])),
            "wo": _b16(Wo[fs, :].reshape(MT, 128, D).transpose(
                1, 0, 2).reshape(128, -1)),
            "bq": np.ascontiguousarray((bq[fs] * SCALE).reshape(MT, 128).T),
        })
    return maps


def kernel(trace=False, **inputs):
    nc = _build()
    res = run_bass_kernel_spmd(nc, _shards(inputs), core_ids=list(range(NCORES)),
                               trace=trace)
    partial = np.stack([r_["out"] for r_ in res.results])  # [8, S, D] bf16
    acc = partial.astype(np.float64).reshape(B, GROUPS, S, D).sum(axis=1)
    acc += (np.asarray(inputs["bv"], np.float64) @
            np.asarray(inputs["Wo"], np.float64) +
            np.asarray(inputs["bo"], np.float64))
    out = acc.astype(np.float32)
    if trace:
        return out, res
    return out
